# revision 34
# baseline (speedup 1.0000x reference)
"""Trainium2 Bass kernel for nn_DistLoss_18949395710456 (retrieval_knn).

Computation (see reference): for each (b, l) stroke pair, gather a "pooled"
color from the ref image at the predicted position, build the L1 color
similarity map over all 256x256 pixels, take the top-8 closest pixels
(exact jax top_k index semantics), convert winners to normalized coords,
distance from stroke l+1's predicted position to stroke l's candidates,
min over the 8 candidates, mean over (b, l=1..127) -> scalar.

Sharding: data-parallel over (b, L): 2 cores per image b, 64 pairs per
core (core 2b: l=0..63; core 2b+1: l=64..126 plus one padded duplicate).
Candidates for l=127 are never used by the loss, so they are not computed.
All arithmetic runs on-device; the host only reindexes inputs (sharding)
and averages the 8 cores' 64-value outputs.

Numerics are bit-exact vs the fp32 reference except:
  - the final /3 of the channel mean is dropped (monotone; verified on the
    fixed input that sum-order == quotient-order for every pair's top-9)
  - the final sqrt runs on the ScalarE LUT (|err| <~1e-6 rel)
Round-half-to-even is done with the 1.5*2^23 magic-add trick; floor(v) for
v = k + m/256 uses rne(v - 127.5/256), both exact in fp32.
"""

import sys

sys.path.insert(0, "/opt/trn_rl_repo")

import numpy as np

import concourse.bass as bass
import concourse.bacc as bacc
import concourse.mybir as mybir
from concourse.bass import IndirectOffsetOnAxis
from concourse.masks import make_identity
from concourse.tile import TileContext

F32 = mybir.dt.float32
U16 = mybir.dt.uint16
U32 = mybir.dt.uint32
ALU = mybir.AluOpType
ACTF = mybir.ActivationFunctionType
AX = mybir.AxisListType

P = 128          # partitions
FD = 512         # free dim: 128*512 = 65536 pixels
NPAIR = 64       # pairs per core
IMG = 256
MAGIC = 12582912.0          # 1.5 * 2^23: rne to integer for |x| < 2^22
FLOOR_BIAS = -0.498046875   # rne(v + this) == floor(v) for v = k + m/256

N_CORES = 8

_cached = {}


def _build_program():
    nc = bacc.Bacc(
        "TRN2",
        target_bir_lowering=False,
        debug=False,
        enable_asserts=False,
        num_devices=N_CORES,
    )
    img = nc.dram_tensor("img", [3, P * FD], F32, kind="ExternalInput").ap()
    gpts = nc.dram_tensor("gpts", [NPAIR, 2], F32, kind="ExternalInput").ap()
    # next-stroke positions prearranged host-side: npx[jj*8+k, c] = x of pair c*16+jj
    npx = nc.dram_tensor("npx", [P, 4], F32, kind="ExternalInput").ap()
    npy = nc.dram_tensor("npy", [P, 4], F32, kind="ExternalInput").ap()
    c512p = nc.dram_tensor("c512p", [P, 1], F32, kind="ExternalInput").ap()
    out = nc.dram_tensor("out", [NPAIR], F32, kind="ExternalOutput").ap()
    probe_out = nc.dram_tensor("probe", [1], F32, kind="ExternalOutput").ap()

    from contextlib import ExitStack

    with TileContext(nc) as tc, ExitStack() as ctx:
        consts = ctx.enter_context(tc.tile_pool(name="consts", bufs=1))
        small = ctx.enter_context(tc.tile_pool(name="small", bufs=6))
        big = ctx.enter_context(tc.tile_pool(name="big", bufs=5))
        keyp = ctx.enter_context(tc.tile_pool(name="keyp", bufs=18))
        psum = ctx.enter_context(tc.tile_pool(name="psum", bufs=2, space="PSUM"))
        psumk = ctx.enter_context(tc.tile_pool(name="psumk", bufs=2, space="PSUM"))
        psum1 = ctx.enter_context(tc.tile_pool(name="psum1", bufs=1, space="PSUM"))

        # ---- one-time setup ----
        ident = consts.tile([P, P], F32)
        make_identity(nc, ident)
        # -I: PE-side negation (out = (-I).T @ x = -x), off the shared
        # DVE/GpSimd SBUF port
        nident = consts.tile([P, P], F32)
        nc.gpsimd.memset(nident[:], 0.0)
        nc.gpsimd.affine_select(
            out=nident[:], in_=nident[:], compare_op=ALU.not_equal,
            fill=-1.0, base=0, pattern=[[-1, P]], channel_multiplier=1,
        )
        mones1 = consts.tile([1, P], F32)
        nc.vector.memset(mones1[:], -1.0)

        cp = consts.tile([P, 1], F32)
        nc.sync.dma_start(out=cp[:], in_=c512p)

        r = []
        for c in range(3):
            rc = consts.tile([P, FD], F32, tag=f"r{c}")
            nc.sync.dma_start(out=rc[:], in_=img[c].rearrange("(p f) -> p f", p=P))
            r.append(rc)

        # grid -> pixel index q per pair
        gp = consts.tile([NPAIR, 2], F32)
        nc.sync.dma_start(out=gp[:], in_=gpts)
        u = consts.tile([NPAIR, 2], F32)
        # u = g*256 - 0.5  (g*256 exact, one rounding for -0.5, same as jax)
        nc.vector.tensor_scalar(u[:], gp[:], 256.0, -0.5, op0=ALU.mult, op1=ALU.add)
        u2 = consts.tile([NPAIR, 2], F32)
        nc.vector.tensor_scalar_add(u2[:], u[:], MAGIC)
        u3 = consts.tile([NPAIR, 2], F32)
        nc.vector.tensor_scalar_sub(u3[:], u2[:], MAGIC)
        uc = consts.tile([NPAIR, 2], F32)
        nc.vector.tensor_scalar(uc[:], u3[:], 0.0, 255.0, op0=ALU.max, op1=ALU.min)
        qf = consts.tile([NPAIR, 1], F32)
        # q = iy*256 + ix (exact: < 2^17)
        nc.vector.scalar_tensor_tensor(
            out=qf[:], in0=uc[:, 1:2], scalar=256.0, in1=uc[:, 0:1],
            op0=ALU.mult, op1=ALU.add,
        )
        qu = consts.tile([NPAIR, 1], U32)
        nc.vector.tensor_copy(out=qu[:], in_=qf[:])

        # gather pooled colors: colors[i, ch] = img[ch, q[i]]
        colors = consts.tile([NPAIR, 3], F32)
        img_flat = img.rearrange("c q -> (c q)")[:, None]
        for ch in range(3):
            nc.gpsimd.indirect_dma_start(
                out=colors[:, ch : ch + 1],
                out_offset=None,
                in_=img_flat,
                in_offset=IndirectOffsetOnAxis(ap=qu[:, :1], axis=0),
                element_offset=ch * P * FD,
            )
        # flatten to one partition: cflat[0, i*3 + ch] = colors[i, ch]
        cflat = consts.tile([1, 3 * NPAIR], F32)
        nc.sync.dma_start(out=cflat[0:1, :], in_=colors[:])
        cbc = consts.tile([P, 3 * NPAIR], F32)
        nc.gpsimd.partition_broadcast(cbc[:], cflat[0:1, :])

        # next-stroke positions, already host-arranged to the chunk layout
        nxb = consts.tile([P, 4], F32)
        nc.sync.dma_start(out=nxb[:], in_=npx)
        nyb = consts.tile([P, 4], F32)
        nc.sync.dma_start(out=nyb[:], in_=npy)

        # all pairs' per-partition winner claims: columns 8i..8i+8 = pair i
        midxall = consts.tile([P, 8 * NPAIR], U16)

        # ---- per-pair pipeline, grouped by 8 pairs per gf-DMA ----

        def stage_a(i):
            a0 = big.tile([P, FD], F32, tag="a0")
            a1 = big.tile([P, FD], F32, tag="a1")
            a2 = big.tile([P, FD], F32, tag="a2")
            # a_ch = |c_ch - ref_ch| == |ref_ch - c_ch|
            nc.scalar.activation(a0[:], r[0][:], ACTF.Abs,
                                 bias=cbc[:, 3 * i + 0 : 3 * i + 1], scale=-1.0)
            nc.scalar.activation(a1[:], r[1][:], ACTF.Abs,
                                 bias=cbc[:, 3 * i + 1 : 3 * i + 2], scale=-1.0)
            nc.scalar.activation(a2[:], r[2][:], ACTF.Abs,
                                 bias=cbc[:, 3 * i + 2 : 3 * i + 3], scale=-1.0)
            t = big.tile([P, FD], F32, tag="t")
            # t = a0 + a1; s = t + a2 (split between DVE and GpSimd —
            # they share an SBUF port, so balance their totals)
            eng_t = nc.gpsimd if i % 2 == 0 else nc.vector
            eng_s = nc.vector if i % 2 == 0 else nc.gpsimd
            eng_t.tensor_tensor(out=t[:], in0=a0[:], in1=a1[:], op=ALU.add)
            s = keyp.tile([P, FD], F32, tag="s")
            eng_s.tensor_tensor(out=s[:], in0=t[:], in1=a2[:], op=ALU.add)
            # negate on the TensorEngine (own SBUF ports): keyP = -s in PSUM
            keyP = psumk.tile([P, FD], F32, tag="keyP")
            nc.tensor.matmul(keyP[:], nident[:], s[:])
            # per-partition top-8 of this pair -> column block of the group tile
            j = i % 8
            nc.vector.max(out=candall[:, 8 * j : 8 * j + 8], in_=keyP[:])
            return s

        def mid_group(g, keys):
            # one transpose for the whole group: (128, 64) -> (64, 128);
            # pair j occupies rows 8j..8j+8
            candTall = psum.tile([NPAIR, P], F32, tag="candTall")
            nc.tensor.transpose(candTall[:], candall[:], ident[:])
            g1b = small.tile([NPAIR, 8], F32, tag="g1b")
            for q in range(0, NPAIR, 32):
                nc.vector.max(out=g1b[q : q + 32, :],
                              in_=candTall[q : q + 32, :])
            gfall = small.tile([1, 512], F32, tag="gfall")
            nc.sync.dma_start(out=gfall[0:1, :], in_=g1b[:])
            return keys, gfall

        def finish_group(g, keys, gfall):
            # global top-8 values per pair (still negated); one PE broadcast
            # for the whole group with -ones flips them to +sim, matching
            # the SBUF sums that max_index scans
            gwin8 = small.tile([1, 64], F32, tag="gwin8")
            for j in range(8):
                nc.vector.max(out=gwin8[0:1, 8 * j : 8 * j + 8],
                              in_=gfall[0:1, 64 * j : 64 * j + 64])
            gwb8 = psum.tile([P, 64], F32, tag="gwb8")
            nc.tensor.matmul(gwb8[:], mones1[:], gwin8[:])
            for j in range(8):
                i = 8 * g + j
                nc.vector.max_index(out=midxall[:, 8 * i : 8 * i + 8],
                                    in_max=gwb8[:, 8 * j : 8 * j + 8],
                                    in_values=keys[j][:])

        pending = None
        for g in range(8):
            candall = small.tile([P, 64], F32, tag="candall")
            keys = [stage_a(8 * g + j) for j in range(8)]
            mid = mid_group(g, keys)
            if pending is not None:
                finish_group(g - 1, *pending)
            pending = mid
        finish_group(7, *pending)

        # ---- batched winner resolution: 4 chunks of 16 pairs ----
        midxf = consts.tile([P, 8 * NPAIR], F32)
        nc.vector.tensor_copy(out=midxf[:], in_=midxall[:])
        flatall = consts.tile([P, 8 * NPAIR], F32)
        nc.vector.tensor_scalar_add(flatall[:], midxf[:], cp[:, 0:1])
        flats = consts.tile([P, 4], F32)
        for c in range(4):
            fT = psum1.tile([P, P], F32, tag="fT")
            nc.tensor.transpose(fT[:], flatall[:, P * c : P * (c + 1)], ident[:])
            # winner flat pixel index (invalid rows sort above 65535)
            nc.vector.tensor_reduce(out=flats[:, c : c + 1], in_=fT[:],
                                    axis=AX.X, op=ALU.min)

        # ---- tail: coords, distances, min over K, sqrt ----
        v = consts.tile([P, 4], F32)
        # v = flat/256 - 127.5/256 (flat/256 exact)
        nc.vector.tensor_scalar(v[:], flats[:], 0.00390625, FLOOR_BIAS,
                                op0=ALU.mult, op1=ALU.add)
        v2 = consts.tile([P, 4], F32)
        nc.vector.tensor_scalar_add(v2[:], v[:], MAGIC)
        yy = consts.tile([P, 4], F32)
        nc.vector.tensor_scalar_sub(yy[:], v2[:], MAGIC)   # yy = flat // 256
        xx = consts.tile([P, 4], F32)
        # xx = flat - 256*yy
        nc.vector.scalar_tensor_tensor(
            out=xx[:], in0=yy[:], scalar=-256.0, in1=flats[:],
            op0=ALU.mult, op1=ALU.add,
        )
        dx = consts.tile([P, 4], F32)
        # dx = nx - xx/256 (xx/256 exact, single rounding on the subtract)
        nc.vector.scalar_tensor_tensor(
            out=dx[:], in0=xx[:], scalar=-0.00390625, in1=nxb[:],
            op0=ALU.mult, op1=ALU.add,
        )
        dy = consts.tile([P, 4], F32)
        nc.vector.scalar_tensor_tensor(
            out=dy[:], in0=yy[:], scalar=-0.00390625, in1=nyb[:],
            op0=ALU.mult, op1=ALU.add,
        )
        dx2 = consts.tile([P, 4], F32)
        nc.vector.tensor_tensor(out=dx2[:], in0=dx[:], in1=dx[:], op=ALU.mult)
        dy2 = consts.tile([P, 4], F32)
        nc.vector.tensor_tensor(out=dy2[:], in0=dy[:], in1=dy[:], op=ALU.mult)
        d2 = consts.tile([P, 4], F32)
        nc.vector.tensor_tensor(out=d2[:], in0=dx2[:], in1=dy2[:], op=ALU.add)
        d2T = psum1.tile([4, P], F32, tag="d2T")
        nc.tensor.transpose(d2T[:], d2[:], ident[:])
        # min over the 8 ranks of each pair: (4, 16, 8) reduce innermost
        md2 = consts.tile([4, 16], F32)
        nc.vector.tensor_reduce(
            out=md2[:], in_=d2T[:].rearrange("c (j k) -> c j k", k=8),
            axis=AX.X, op=ALU.min,
        )
        val = consts.tile([4, 16], F32)
        nc.scalar.activation(val[:], md2[:], ACTF.Sqrt)
        nc.sync.dma_start(out=out.rearrange("(c j) -> c j", c=4), in_=val[:])
        nc.sync.dma_start(out=probe_out, in_=val[0:1, 0])

    nc.compile()
    return nc


def _get_program():
    if "nc" not in _cached:
        _cached["nc"] = _build_program()
    return _cached["nc"]


def make_in_maps(predictions: np.ndarray, ref_imgs: np.ndarray):
    """Shard full inputs into 8 per-core input dicts (pure reindexing)."""
    bs, L, _ = predictions.shape
    pp = predictions[:, :, :2]
    grid = np.ascontiguousarray(pp.reshape(bs * L, 2))
    c512p = (np.arange(P, dtype=np.float32) * FD).reshape(P, 1)
    in_maps = []
    for core in range(N_CORES):
        b = core // 2
        if core % 2 == 0:
            ls = list(range(0, 64))
        else:
            ls = list(range(64, 127)) + [126]  # 63 real pairs + 1 pad
        fi = [l * bs + b for l in ls]
        nxt = pp[b, [l + 1 for l in ls]]  # (64, 2), pair order
        # chunk layout: npx[jj*8+k, c] = x of pair c*16+jj (k = rank, repeated)
        npx = np.repeat(nxt[:, 0].reshape(4, 16), 8, axis=1).reshape(4, 128).T
        npy = np.repeat(nxt[:, 1].reshape(4, 16), 8, axis=1).reshape(4, 128).T
        in_maps.append({
            "img": np.ascontiguousarray(ref_imgs[b].reshape(3, P * FD)),
            "gpts": np.ascontiguousarray(grid[fi]),
            "npx": np.ascontiguousarray(npx.astype(np.float32)),
            "npy": np.ascontiguousarray(npy.astype(np.float32)),
            "c512p": c512p,
        })
    return in_maps


def kernel(predictions: np.ndarray, ref_imgs: np.ndarray) -> np.ndarray:
    from concourse.bass_utils import run_bass_kernel_spmd

    nc = _get_program()
    in_maps = make_in_maps(predictions, ref_imgs)
    res = run_bass_kernel_spmd(nc, in_maps, core_ids=list(range(N_CORES)))
    rows = []
    for b in range(4):
        rows.append(np.concatenate([
            res.results[2 * b]["out"][:64],
            res.results[2 * b + 1]["out"][:63],
        ]))
    val_down = np.stack(rows)  # (4, 127)
    return np.float32(np.mean(val_down))


# revision 37
# speedup vs baseline: 1.0711x; 1.0711x over previous
"""Trainium2 Bass kernel for nn_DistLoss_18949395710456 (retrieval_knn).

Computation (see reference): for each (b, l) stroke pair, gather a "pooled"
color from the ref image at the predicted position, build the L1 color
similarity map over all 256x256 pixels, take the top-8 closest pixels
(exact jax top_k index semantics), convert winners to normalized coords,
distance from stroke l+1's predicted position to stroke l's candidates,
min over the 8 candidates, mean over (b, l=1..127) -> scalar.

Sharding: data-parallel over (b, L): 2 cores per image b, 64 pairs per
core (core 2b: l=0..63; core 2b+1: l=64..126 plus one padded duplicate).
Candidates for l=127 are never used by the loss, so they are not computed.
All arithmetic runs on-device; the host only reindexes inputs (sharding)
and averages the 8 cores' 64-value outputs.

Numerics are bit-exact vs the fp32 reference except:
  - the final /3 of the channel mean is dropped (monotone; verified on the
    fixed input that sum-order == quotient-order for every pair's top-9)
  - the final sqrt runs on the ScalarE LUT (|err| <~1e-6 rel)
Round-half-to-even is done with the 1.5*2^23 magic-add trick; floor(v) for
v = k + m/256 uses rne(v - 127.5/256), both exact in fp32.
"""

import sys

sys.path.insert(0, "/opt/trn_rl_repo")

import numpy as np

import concourse.bass as bass
import concourse.bacc as bacc
import concourse.mybir as mybir
from concourse.bass import IndirectOffsetOnAxis
from concourse.masks import make_identity
from concourse.tile import TileContext

F32 = mybir.dt.float32
U16 = mybir.dt.uint16
U32 = mybir.dt.uint32
ALU = mybir.AluOpType
ACTF = mybir.ActivationFunctionType
AX = mybir.AxisListType

P = 128          # partitions
FD = 512         # free dim: 128*512 = 65536 pixels
NPAIR = 64       # pairs per core
IMG = 256
MAGIC = 12582912.0          # 1.5 * 2^23: rne to integer for |x| < 2^22
FLOOR_BIAS = -0.498046875   # rne(v + this) == floor(v) for v = k + m/256

N_CORES = 8

_cached = {}


def _build_program():
    nc = bacc.Bacc(
        "TRN2",
        target_bir_lowering=False,
        debug=False,
        enable_asserts=False,
        num_devices=N_CORES,
    )
    img = nc.dram_tensor("img", [3, P * FD], F32, kind="ExternalInput").ap()
    gpts = nc.dram_tensor("gpts", [NPAIR, 2], F32, kind="ExternalInput").ap()
    # next-stroke positions prearranged host-side: npx[jj*8+k, c] = x of pair c*16+jj
    npx = nc.dram_tensor("npx", [P, 4], F32, kind="ExternalInput").ap()
    npy = nc.dram_tensor("npy", [P, 4], F32, kind="ExternalInput").ap()
    c512p = nc.dram_tensor("c512p", [P, 1], F32, kind="ExternalInput").ap()
    out = nc.dram_tensor("out", [NPAIR], F32, kind="ExternalOutput").ap()
    probe_out = nc.dram_tensor("probe", [1], F32, kind="ExternalOutput").ap()

    from contextlib import ExitStack

    with TileContext(nc) as tc, ExitStack() as ctx:
        consts = ctx.enter_context(tc.tile_pool(name="consts", bufs=1))
        small = ctx.enter_context(tc.tile_pool(name="small", bufs=6))
        big = ctx.enter_context(tc.tile_pool(name="big", bufs=5))
        keyp = ctx.enter_context(tc.tile_pool(name="keyp", bufs=18))
        psum = ctx.enter_context(tc.tile_pool(name="psum", bufs=3, space="PSUM"))
        psum1 = ctx.enter_context(tc.tile_pool(name="psum1", bufs=1, space="PSUM"))

        # ---- one-time setup ----
        ident = consts.tile([P, P], F32)
        make_identity(nc, ident)
        ones1 = consts.tile([1, P], F32)
        nc.vector.memset(ones1[:], 1.0)

        cp = consts.tile([P, 1], F32)
        nc.sync.dma_start(out=cp[:], in_=c512p)

        r = []
        for c in range(3):
            rc = consts.tile([P, FD], F32, tag=f"r{c}")
            nc.sync.dma_start(out=rc[:], in_=img[c].rearrange("(p f) -> p f", p=P))
            r.append(rc)

        # grid -> pixel index q per pair
        gp = consts.tile([NPAIR, 2], F32)
        nc.sync.dma_start(out=gp[:], in_=gpts)
        u = consts.tile([NPAIR, 2], F32)
        # u = g*256 - 0.5  (g*256 exact, one rounding for -0.5, same as jax)
        nc.vector.tensor_scalar(u[:], gp[:], 256.0, -0.5, op0=ALU.mult, op1=ALU.add)
        u2 = consts.tile([NPAIR, 2], F32)
        nc.vector.tensor_scalar_add(u2[:], u[:], MAGIC)
        u3 = consts.tile([NPAIR, 2], F32)
        nc.vector.tensor_scalar_sub(u3[:], u2[:], MAGIC)
        uc = consts.tile([NPAIR, 2], F32)
        nc.vector.tensor_scalar(uc[:], u3[:], 0.0, 255.0, op0=ALU.max, op1=ALU.min)
        qf = consts.tile([NPAIR, 1], F32)
        # q = iy*256 + ix (exact: < 2^17)
        nc.vector.scalar_tensor_tensor(
            out=qf[:], in0=uc[:, 1:2], scalar=256.0, in1=uc[:, 0:1],
            op0=ALU.mult, op1=ALU.add,
        )
        qu = consts.tile([NPAIR, 1], U32)
        nc.vector.tensor_copy(out=qu[:], in_=qf[:])

        # gather pooled colors: colors[i, ch] = img[ch, q[i]]
        colors = consts.tile([NPAIR, 3], F32)
        img_flat = img.rearrange("c q -> (c q)")[:, None]
        for ch in range(3):
            nc.gpsimd.indirect_dma_start(
                out=colors[:, ch : ch + 1],
                out_offset=None,
                in_=img_flat,
                in_offset=IndirectOffsetOnAxis(ap=qu[:, :1], axis=0),
                element_offset=ch * P * FD,
            )
        # flatten to one partition: cflat[0, i*3 + ch] = colors[i, ch]
        cflat = consts.tile([1, 3 * NPAIR], F32)
        nc.sync.dma_start(out=cflat[0:1, :], in_=colors[:])
        cbc = consts.tile([P, 3 * NPAIR], F32)
        nc.gpsimd.partition_broadcast(cbc[:], cflat[0:1, :])

        # next-stroke positions, already host-arranged to the chunk layout
        nxb = consts.tile([P, 4], F32)
        nc.sync.dma_start(out=nxb[:], in_=npx)
        nyb = consts.tile([P, 4], F32)
        nc.sync.dma_start(out=nyb[:], in_=npy)

        # all pairs' per-partition winner claims: columns 8i..8i+8 = pair i
        midxall = consts.tile([P, 8 * NPAIR], U16)

        # ---- per-pair pipeline, grouped by 8 pairs per gf-DMA ----

        def stage_a(i):
            a0 = big.tile([P, FD], F32, tag="a0")
            a1 = big.tile([P, FD], F32, tag="a1")
            a2 = big.tile([P, FD], F32, tag="a2")
            # a_ch = |c_ch - ref_ch| == |ref_ch - c_ch|
            nc.scalar.activation(a0[:], r[0][:], ACTF.Abs,
                                 bias=cbc[:, 3 * i + 0 : 3 * i + 1], scale=-1.0)
            nc.scalar.activation(a1[:], r[1][:], ACTF.Abs,
                                 bias=cbc[:, 3 * i + 1 : 3 * i + 2], scale=-1.0)
            nc.scalar.activation(a2[:], r[2][:], ACTF.Abs,
                                 bias=cbc[:, 3 * i + 2 : 3 * i + 3], scale=-1.0)
            t = big.tile([P, FD], F32, tag="t")
            # t = a0 + a1 (always gpsimd)
            nc.gpsimd.tensor_tensor(out=t[:], in0=a0[:], in1=a1[:], op=ALU.add)
            key = keyp.tile([P, FD], F32, tag="key")
            # key = -((a0+a1)+a2): top-8 of key == top-8 of -sim.
            # Three engine placements, balanced empirically: DVE and GpSimd
            # share an SBUF port, ACT has its own.
            if i % 2 == 0:
                # fused add+negate on DVE
                nc.vector.scalar_tensor_tensor(
                    out=key[:], in0=a2[:], scalar=-1.0, in1=t[:],
                    op0=ALU.mult, op1=ALU.subtract,
                )
            else:
                s = big.tile([P, FD], F32, tag="s")
                nc.gpsimd.tensor_tensor(out=s[:], in0=t[:], in1=a2[:], op=ALU.add)
                if i % 8 in (1, 3, 5):
                    # exact negate on ScalarE: Copy(-1*s + 0)
                    nc.scalar.activation(key[:], s[:], ACTF.Copy, scale=-1.0)
                else:
                    nc.vector.tensor_scalar_mul(key[:], s[:], -1.0)
            # per-partition top-8 of this pair -> column block of the group tile
            j = i % 8
            nc.vector.max(out=candall[:, 8 * j : 8 * j + 8], in_=key[:])
            return key

        def mid_group(g, keys):
            # one transpose for the whole group: (128, 64) -> (64, 128);
            # pair j occupies rows 8j..8j+8
            candTall = psum.tile([NPAIR, P], F32, tag="candTall")
            nc.tensor.transpose(candTall[:], candall[:], ident[:])
            g1b = small.tile([NPAIR, 8], F32, tag="g1b")
            for q in range(0, NPAIR, 32):
                nc.vector.max(out=g1b[q : q + 32, :],
                              in_=candTall[q : q + 32, :])
            gfall = small.tile([1, 512], F32, tag="gfall")
            nc.sync.dma_start(out=gfall[0:1, :], in_=g1b[:])
            return keys, gfall

        def finish_group(g, keys, gfall):
            # global top-8 values per pair (still negated); one PE broadcast
            # for the whole group with -ones flips them to +sim, matching
            # the SBUF sums that max_index scans
            gwin8 = small.tile([1, 64], F32, tag="gwin8")
            for j in range(8):
                nc.vector.max(out=gwin8[0:1, 8 * j : 8 * j + 8],
                              in_=gfall[0:1, 64 * j : 64 * j + 64])
            gwb8 = psum.tile([P, 64], F32, tag="gwb8")
            nc.tensor.matmul(gwb8[:], ones1[:], gwin8[:])
            for j in range(8):
                i = 8 * g + j
                nc.vector.max_index(out=midxall[:, 8 * i : 8 * i + 8],
                                    in_max=gwb8[:, 8 * j : 8 * j + 8],
                                    in_values=keys[j][:])

        pending = None
        for g in range(8):
            candall = small.tile([P, 64], F32, tag="candall")
            keys = [stage_a(8 * g + j) for j in range(8)]
            mid = mid_group(g, keys)
            if pending is not None:
                finish_group(g - 1, *pending)
            pending = mid
        finish_group(7, *pending)

        # ---- batched winner resolution: 4 chunks of 16 pairs ----
        midxf = consts.tile([P, 8 * NPAIR], F32)
        nc.vector.tensor_copy(out=midxf[:], in_=midxall[:])
        flatall = consts.tile([P, 8 * NPAIR], F32)
        nc.vector.tensor_scalar_add(flatall[:], midxf[:], cp[:, 0:1])
        flats = consts.tile([P, 4], F32)
        for c in range(4):
            fT = psum1.tile([P, P], F32, tag="fT")
            nc.tensor.transpose(fT[:], flatall[:, P * c : P * (c + 1)], ident[:])
            # winner flat pixel index (invalid rows sort above 65535)
            nc.vector.tensor_reduce(out=flats[:, c : c + 1], in_=fT[:],
                                    axis=AX.X, op=ALU.min)

        # ---- tail: coords, distances, min over K, sqrt ----
        v = consts.tile([P, 4], F32)
        # v = flat/256 - 127.5/256 (flat/256 exact)
        nc.vector.tensor_scalar(v[:], flats[:], 0.00390625, FLOOR_BIAS,
                                op0=ALU.mult, op1=ALU.add)
        v2 = consts.tile([P, 4], F32)
        nc.vector.tensor_scalar_add(v2[:], v[:], MAGIC)
        yy = consts.tile([P, 4], F32)
        nc.vector.tensor_scalar_sub(yy[:], v2[:], MAGIC)   # yy = flat // 256
        xx = consts.tile([P, 4], F32)
        # xx = flat - 256*yy
        nc.vector.scalar_tensor_tensor(
            out=xx[:], in0=yy[:], scalar=-256.0, in1=flats[:],
            op0=ALU.mult, op1=ALU.add,
        )
        dx = consts.tile([P, 4], F32)
        # dx = nx - xx/256 (xx/256 exact, single rounding on the subtract)
        nc.vector.scalar_tensor_tensor(
            out=dx[:], in0=xx[:], scalar=-0.00390625, in1=nxb[:],
            op0=ALU.mult, op1=ALU.add,
        )
        dy = consts.tile([P, 4], F32)
        nc.vector.scalar_tensor_tensor(
            out=dy[:], in0=yy[:], scalar=-0.00390625, in1=nyb[:],
            op0=ALU.mult, op1=ALU.add,
        )
        dx2 = consts.tile([P, 4], F32)
        nc.vector.tensor_tensor(out=dx2[:], in0=dx[:], in1=dx[:], op=ALU.mult)
        dy2 = consts.tile([P, 4], F32)
        nc.vector.tensor_tensor(out=dy2[:], in0=dy[:], in1=dy[:], op=ALU.mult)
        d2 = consts.tile([P, 4], F32)
        nc.vector.tensor_tensor(out=d2[:], in0=dx2[:], in1=dy2[:], op=ALU.add)
        d2T = psum1.tile([4, P], F32, tag="d2T")
        nc.tensor.transpose(d2T[:], d2[:], ident[:])
        # min over the 8 ranks of each pair: (4, 16, 8) reduce innermost
        md2 = consts.tile([4, 16], F32)
        nc.vector.tensor_reduce(
            out=md2[:], in_=d2T[:].rearrange("c (j k) -> c j k", k=8),
            axis=AX.X, op=ALU.min,
        )
        val = consts.tile([4, 16], F32)
        nc.scalar.activation(val[:], md2[:], ACTF.Sqrt)
        nc.sync.dma_start(out=out.rearrange("(c j) -> c j", c=4), in_=val[:])
        nc.sync.dma_start(out=probe_out, in_=val[0:1, 0])

    nc.compile()
    return nc


def _get_program():
    if "nc" not in _cached:
        _cached["nc"] = _build_program()
    return _cached["nc"]


def make_in_maps(predictions: np.ndarray, ref_imgs: np.ndarray):
    """Shard full inputs into 8 per-core input dicts (pure reindexing)."""
    bs, L, _ = predictions.shape
    pp = predictions[:, :, :2]
    grid = np.ascontiguousarray(pp.reshape(bs * L, 2))
    c512p = (np.arange(P, dtype=np.float32) * FD).reshape(P, 1)
    in_maps = []
    for core in range(N_CORES):
        b = core // 2
        if core % 2 == 0:
            ls = list(range(0, 64))
        else:
            ls = list(range(64, 127)) + [126]  # 63 real pairs + 1 pad
        fi = [l * bs + b for l in ls]
        nxt = pp[b, [l + 1 for l in ls]]  # (64, 2), pair order
        # chunk layout: npx[jj*8+k, c] = x of pair c*16+jj (k = rank, repeated)
        npx = np.repeat(nxt[:, 0].reshape(4, 16), 8, axis=1).reshape(4, 128).T
        npy = np.repeat(nxt[:, 1].reshape(4, 16), 8, axis=1).reshape(4, 128).T
        in_maps.append({
            "img": np.ascontiguousarray(ref_imgs[b].reshape(3, P * FD)),
            "gpts": np.ascontiguousarray(grid[fi]),
            "npx": np.ascontiguousarray(npx.astype(np.float32)),
            "npy": np.ascontiguousarray(npy.astype(np.float32)),
            "c512p": c512p,
        })
    return in_maps


def kernel(predictions: np.ndarray, ref_imgs: np.ndarray) -> np.ndarray:
    from concourse.bass_utils import run_bass_kernel_spmd

    nc = _get_program()
    in_maps = make_in_maps(predictions, ref_imgs)
    res = run_bass_kernel_spmd(nc, in_maps, core_ids=list(range(N_CORES)))
    rows = []
    for b in range(4):
        rows.append(np.concatenate([
            res.results[2 * b]["out"][:64],
            res.results[2 * b + 1]["out"][:63],
        ]))
    val_down = np.stack(rows)  # (4, 127)
    return np.float32(np.mean(val_down))


# revision 39
# speedup vs baseline: 1.1731x; 1.0952x over previous
"""Trainium2 Bass kernel for nn_DistLoss_18949395710456 (retrieval_knn).

Computation (see reference): for each (b, l) stroke pair, gather a "pooled"
color from the ref image at the predicted position, build the L1 color
similarity map over all 256x256 pixels, take the top-8 closest pixels
(exact jax top_k index semantics), convert winners to normalized coords,
distance from stroke l+1's predicted position to stroke l's candidates,
min over the 8 candidates, mean over (b, l=1..127) -> scalar.

Sharding: data-parallel over (b, L): 2 cores per image b, 64 pairs per
core (core 2b: l=0..63; core 2b+1: l=64..126 plus one padded duplicate).
Candidates for l=127 are never used by the loss, so they are not computed.
All arithmetic runs on-device; the host only reindexes inputs (sharding)
and averages the 8 cores' 64-value outputs.

Numerics are bit-exact vs the fp32 reference except:
  - the final /3 of the channel mean is dropped (monotone; verified on the
    fixed input that sum-order == quotient-order for every pair's top-9)
  - the final sqrt runs on the ScalarE LUT (|err| <~1e-6 rel)
Round-half-to-even is done with the 1.5*2^23 magic-add trick; floor(v) for
v = k + m/256 uses rne(v - 127.5/256), both exact in fp32.
"""

import sys

sys.path.insert(0, "/opt/trn_rl_repo")

import numpy as np

import concourse.bass as bass
import concourse.bacc as bacc
import concourse.mybir as mybir
from concourse.bass import IndirectOffsetOnAxis
from concourse.masks import make_identity
from concourse.tile import TileContext

F32 = mybir.dt.float32
U16 = mybir.dt.uint16
U32 = mybir.dt.uint32
ALU = mybir.AluOpType
ACTF = mybir.ActivationFunctionType
AX = mybir.AxisListType

P = 128          # partitions
FD = 512         # free dim: 128*512 = 65536 pixels
NPAIR = 64       # pairs per core
IMG = 256
MAGIC = 12582912.0          # 1.5 * 2^23: rne to integer for |x| < 2^22
FLOOR_BIAS = -0.498046875   # rne(v + this) == floor(v) for v = k + m/256

N_CORES = 8

_cached = {}


def _build_program():
    nc = bacc.Bacc(
        "TRN2",
        target_bir_lowering=False,
        debug=False,
        enable_asserts=False,
        num_devices=N_CORES,
    )
    img = nc.dram_tensor("img", [3, P * FD], F32, kind="ExternalInput").ap()
    gpts = nc.dram_tensor("gpts", [NPAIR, 2], F32, kind="ExternalInput").ap()
    # next-stroke positions prearranged host-side: npx[jj*8+k, c] = x of pair c*16+jj
    npx = nc.dram_tensor("npx", [P, 4], F32, kind="ExternalInput").ap()
    npy = nc.dram_tensor("npy", [P, 4], F32, kind="ExternalInput").ap()
    c512p = nc.dram_tensor("c512p", [P, 1], F32, kind="ExternalInput").ap()
    out = nc.dram_tensor("out", [NPAIR], F32, kind="ExternalOutput").ap()
    probe_out = nc.dram_tensor("probe", [1], F32, kind="ExternalOutput").ap()

    from contextlib import ExitStack

    with TileContext(nc) as tc, ExitStack() as ctx:
        consts = ctx.enter_context(tc.tile_pool(name="consts", bufs=1))
        small = ctx.enter_context(tc.tile_pool(name="small", bufs=6))
        big = ctx.enter_context(tc.tile_pool(name="big", bufs=5))
        keyp = ctx.enter_context(tc.tile_pool(name="keyp", bufs=18))
        psum = ctx.enter_context(tc.tile_pool(name="psum", bufs=3, space="PSUM"))
        psum1 = ctx.enter_context(tc.tile_pool(name="psum1", bufs=1, space="PSUM"))

        # ---- one-time setup ----
        ident = consts.tile([P, P], F32)
        make_identity(nc, ident)
        ones1 = consts.tile([1, P], F32)
        nc.vector.memset(ones1[:], 1.0)

        cp = consts.tile([P, 1], F32)
        nc.sync.dma_start(out=cp[:], in_=c512p)

        r = []
        for c in range(3):
            rc = consts.tile([P, FD], F32, tag=f"r{c}")
            nc.sync.dma_start(out=rc[:], in_=img[c].rearrange("(p f) -> p f", p=P))
            r.append(rc)

        # grid -> pixel index q per pair
        gp = consts.tile([NPAIR, 2], F32)
        nc.sync.dma_start(out=gp[:], in_=gpts)
        u = consts.tile([NPAIR, 2], F32)
        # u = g*256 - 0.5  (g*256 exact, one rounding for -0.5, same as jax)
        nc.vector.tensor_scalar(u[:], gp[:], 256.0, -0.5, op0=ALU.mult, op1=ALU.add)
        u2 = consts.tile([NPAIR, 2], F32)
        nc.vector.tensor_scalar_add(u2[:], u[:], MAGIC)
        u3 = consts.tile([NPAIR, 2], F32)
        nc.vector.tensor_scalar_sub(u3[:], u2[:], MAGIC)
        uc = consts.tile([NPAIR, 2], F32)
        nc.vector.tensor_scalar(uc[:], u3[:], 0.0, 255.0, op0=ALU.max, op1=ALU.min)
        qf = consts.tile([NPAIR, 1], F32)
        # q = iy*256 + ix (exact: < 2^17)
        nc.vector.scalar_tensor_tensor(
            out=qf[:], in0=uc[:, 1:2], scalar=256.0, in1=uc[:, 0:1],
            op0=ALU.mult, op1=ALU.add,
        )
        qu = consts.tile([NPAIR, 1], U32)
        nc.vector.tensor_copy(out=qu[:], in_=qf[:])

        # gather pooled colors: colors[i, ch] = img[ch, q[i]]
        colors = consts.tile([NPAIR, 3], F32)
        img_flat = img.rearrange("c q -> (c q)")[:, None]
        for ch in range(3):
            nc.gpsimd.indirect_dma_start(
                out=colors[:, ch : ch + 1],
                out_offset=None,
                in_=img_flat,
                in_offset=IndirectOffsetOnAxis(ap=qu[:, :1], axis=0),
                element_offset=ch * P * FD,
            )
        # flatten to one partition: cflat[0, i*3 + ch] = colors[i, ch]
        cflat = consts.tile([1, 3 * NPAIR], F32)
        nc.sync.dma_start(out=cflat[0:1, :], in_=colors[:])
        cbc = consts.tile([P, 3 * NPAIR], F32)
        nc.gpsimd.partition_broadcast(cbc[:], cflat[0:1, :])

        # next-stroke positions, already host-arranged to the chunk layout
        nxb = consts.tile([P, 4], F32)
        nc.sync.dma_start(out=nxb[:], in_=npx)
        nyb = consts.tile([P, 4], F32)
        nc.sync.dma_start(out=nyb[:], in_=npy)

        # all pairs' per-partition winner claims: columns 8i..8i+8 = pair i
        midxall = consts.tile([P, 8 * NPAIR], U16)

        # ---- per-pair pipeline, grouped by 8 pairs per gf-DMA ----

        def stage_a(i):
            a0 = big.tile([P, FD], F32, tag="a0")
            a1 = big.tile([P, FD], F32, tag="a1")
            a2 = big.tile([P, FD], F32, tag="a2")
            # a_ch = |c_ch - ref_ch| == |ref_ch - c_ch|
            nc.scalar.activation(a0[:], r[0][:], ACTF.Abs,
                                 bias=cbc[:, 3 * i + 0 : 3 * i + 1], scale=-1.0)
            nc.scalar.activation(a1[:], r[1][:], ACTF.Abs,
                                 bias=cbc[:, 3 * i + 1 : 3 * i + 2], scale=-1.0)
            nc.scalar.activation(a2[:], r[2][:], ACTF.Abs,
                                 bias=cbc[:, 3 * i + 2 : 3 * i + 3], scale=-1.0)
            t = big.tile([P, FD], F32, tag="t")
            # t = a0 + a1 (always gpsimd)
            nc.gpsimd.tensor_tensor(out=t[:], in0=a0[:], in1=a1[:], op=ALU.add)
            key = keyp.tile([P, FD], F32, tag="key")
            # key = -((a0+a1)+a2): top-8 of key == top-8 of -sim.
            # Three engine placements, balanced empirically: DVE and GpSimd
            # share an SBUF port, ACT has its own.
            if i % 2 == 0:
                # fused add+negate on DVE
                nc.vector.scalar_tensor_tensor(
                    out=key[:], in0=a2[:], scalar=-1.0, in1=t[:],
                    op0=ALU.mult, op1=ALU.subtract,
                )
            else:
                s = big.tile([P, FD], F32, tag="s")
                nc.gpsimd.tensor_tensor(out=s[:], in0=t[:], in1=a2[:], op=ALU.add)
                nc.vector.tensor_scalar_mul(key[:], s[:], -1.0)
            # per-partition top-8 of this pair -> column block of the group tile
            j = i % 8
            nc.vector.max(out=candall[:, 8 * j : 8 * j + 8], in_=key[:])
            return key

        def mid_group(g, keys):
            # one transpose for the whole group: (128, 64) -> (64, 128);
            # pair j occupies rows 8j..8j+8
            candTall = psum.tile([NPAIR, P], F32, tag="candTall")
            nc.tensor.transpose(candTall[:], candall[:], ident[:])
            g1b = small.tile([NPAIR, 8], F32, tag="g1b")
            for q in range(0, NPAIR, 32):
                nc.vector.max(out=g1b[q : q + 32, :],
                              in_=candTall[q : q + 32, :])
            gfall = small.tile([1, 512], F32, tag="gfall")
            nc.sync.dma_start(out=gfall[0:1, :], in_=g1b[:])
            return keys, gfall

        def finish_group(g, keys, gfall):
            # for each pair of group g: global top-8 values, broadcast via
            # PE (ones^T @ gwin -> PSUM), then index recovery
            prev = None
            for j in range(8):
                i = 8 * g + j
                gwin = small.tile([1, 8], F32, tag="gwin")
                nc.vector.max(out=gwin[:], in_=gfall[0:1, 64 * j : 64 * j + 64])
                gwb = psum.tile([P, 8], F32, tag="gwb")
                nc.tensor.matmul(gwb[:], ones1[:], gwin[:])
                if prev is not None:
                    pi, pkey, pgwb = prev
                    nc.vector.max_index(out=midxall[:, 8 * pi : 8 * pi + 8],
                                        in_max=pgwb[:], in_values=pkey[:])
                prev = (i, keys[j], gwb)
            pi, pkey, pgwb = prev
            nc.vector.max_index(out=midxall[:, 8 * pi : 8 * pi + 8],
                                in_max=pgwb[:], in_values=pkey[:])

        pending = None
        for g in range(8):
            candall = small.tile([P, 64], F32, tag="candall")
            keys = [stage_a(8 * g + j) for j in range(8)]
            mid = mid_group(g, keys)
            if pending is not None:
                finish_group(g - 1, *pending)
            pending = mid
        finish_group(7, *pending)

        # ---- batched winner resolution: 4 chunks of 16 pairs ----
        midxf = consts.tile([P, 8 * NPAIR], F32)
        nc.vector.tensor_copy(out=midxf[:], in_=midxall[:])
        flatall = consts.tile([P, 8 * NPAIR], F32)
        nc.vector.tensor_scalar_add(flatall[:], midxf[:], cp[:, 0:1])
        flats = consts.tile([P, 4], F32)
        for c in range(4):
            fT = psum1.tile([P, P], F32, tag="fT")
            nc.tensor.transpose(fT[:], flatall[:, P * c : P * (c + 1)], ident[:])
            # winner flat pixel index (invalid rows sort above 65535)
            nc.vector.tensor_reduce(out=flats[:, c : c + 1], in_=fT[:],
                                    axis=AX.X, op=ALU.min)

        # ---- tail: coords, distances, min over K, sqrt ----
        v = consts.tile([P, 4], F32)
        # v = flat/256 - 127.5/256 (flat/256 exact)
        nc.vector.tensor_scalar(v[:], flats[:], 0.00390625, FLOOR_BIAS,
                                op0=ALU.mult, op1=ALU.add)
        v2 = consts.tile([P, 4], F32)
        nc.vector.tensor_scalar_add(v2[:], v[:], MAGIC)
        yy = consts.tile([P, 4], F32)
        nc.vector.tensor_scalar_sub(yy[:], v2[:], MAGIC)   # yy = flat // 256
        xx = consts.tile([P, 4], F32)
        # xx = flat - 256*yy
        nc.vector.scalar_tensor_tensor(
            out=xx[:], in0=yy[:], scalar=-256.0, in1=flats[:],
            op0=ALU.mult, op1=ALU.add,
        )
        dx = consts.tile([P, 4], F32)
        # dx = nx - xx/256 (xx/256 exact, single rounding on the subtract)
        nc.vector.scalar_tensor_tensor(
            out=dx[:], in0=xx[:], scalar=-0.00390625, in1=nxb[:],
            op0=ALU.mult, op1=ALU.add,
        )
        dy = consts.tile([P, 4], F32)
        nc.vector.scalar_tensor_tensor(
            out=dy[:], in0=yy[:], scalar=-0.00390625, in1=nyb[:],
            op0=ALU.mult, op1=ALU.add,
        )
        dx2 = consts.tile([P, 4], F32)
        nc.vector.tensor_tensor(out=dx2[:], in0=dx[:], in1=dx[:], op=ALU.mult)
        dy2 = consts.tile([P, 4], F32)
        nc.vector.tensor_tensor(out=dy2[:], in0=dy[:], in1=dy[:], op=ALU.mult)
        d2 = consts.tile([P, 4], F32)
        nc.vector.tensor_tensor(out=d2[:], in0=dx2[:], in1=dy2[:], op=ALU.add)
        d2T = psum1.tile([4, P], F32, tag="d2T")
        nc.tensor.transpose(d2T[:], d2[:], ident[:])
        # min over the 8 ranks of each pair: (4, 16, 8) reduce innermost
        md2 = consts.tile([4, 16], F32)
        nc.vector.tensor_reduce(
            out=md2[:], in_=d2T[:].rearrange("c (j k) -> c j k", k=8),
            axis=AX.X, op=ALU.min,
        )
        val = consts.tile([4, 16], F32)
        nc.scalar.activation(val[:], md2[:], ACTF.Sqrt)
        nc.sync.dma_start(out=out.rearrange("(c j) -> c j", c=4), in_=val[:])
        nc.sync.dma_start(out=probe_out, in_=val[0:1, 0])

    nc.compile()
    return nc


def _get_program():
    if "nc" not in _cached:
        _cached["nc"] = _build_program()
    return _cached["nc"]


def make_in_maps(predictions: np.ndarray, ref_imgs: np.ndarray):
    """Shard full inputs into 8 per-core input dicts (pure reindexing)."""
    bs, L, _ = predictions.shape
    pp = predictions[:, :, :2]
    grid = np.ascontiguousarray(pp.reshape(bs * L, 2))
    c512p = (np.arange(P, dtype=np.float32) * FD).reshape(P, 1)
    in_maps = []
    for core in range(N_CORES):
        b = core // 2
        if core % 2 == 0:
            ls = list(range(0, 64))
        else:
            ls = list(range(64, 127)) + [126]  # 63 real pairs + 1 pad
        fi = [l * bs + b for l in ls]
        nxt = pp[b, [l + 1 for l in ls]]  # (64, 2), pair order
        # chunk layout: npx[jj*8+k, c] = x of pair c*16+jj (k = rank, repeated)
        npx = np.repeat(nxt[:, 0].reshape(4, 16), 8, axis=1).reshape(4, 128).T
        npy = np.repeat(nxt[:, 1].reshape(4, 16), 8, axis=1).reshape(4, 128).T
        in_maps.append({
            "img": np.ascontiguousarray(ref_imgs[b].reshape(3, P * FD)),
            "gpts": np.ascontiguousarray(grid[fi]),
            "npx": np.ascontiguousarray(npx.astype(np.float32)),
            "npy": np.ascontiguousarray(npy.astype(np.float32)),
            "c512p": c512p,
        })
    return in_maps


def kernel(predictions: np.ndarray, ref_imgs: np.ndarray) -> np.ndarray:
    from concourse.bass_utils import run_bass_kernel_spmd

    nc = _get_program()
    in_maps = make_in_maps(predictions, ref_imgs)
    res = run_bass_kernel_spmd(nc, in_maps, core_ids=list(range(N_CORES)))
    rows = []
    for b in range(4):
        rows.append(np.concatenate([
            res.results[2 * b]["out"][:64],
            res.results[2 * b + 1]["out"][:63],
        ]))
    val_down = np.stack(rows)  # (4, 127)
    return np.float32(np.mean(val_down))
